# revision 14
# baseline (speedup 1.0000x reference)
"""Multi-head attention Bass kernel for Trainium2, sharded over 8 NeuronCores.

Sharding: core c handles batch b = c//4 and head-group g = c%4 (4 of 16 heads,
i.e. a 256-wide slice of the QKV projection output).  Each core computes its
heads' attention and a partial output projection (contribution of its 256
ctx columns to the full [S, D] output).  The host sums the 4 partials per
batch and adds the output bias.

Device-side layout choices:
  - activations shipped pre-transposed: xT = x.T  [D, S] so the contraction
    dim (D) lands on SBUF partitions without any on-device transpose.
  - scores are computed transposed (scoresT[sk, sq]) so the attention weights
    leave softmax with sk on partitions — the contraction layout attn@V needs.
  - softmax denominator comes free from a ones-column appended to V
    (ctx psum row 64 = sum_sk attn);  no max-subtraction (scores bounded).
  - masking is a multiply by a 0/1 bf16 keep-mask after exp.

v2 pipeline (vs the 368us baseline whose PE ran cold and stalled on psum):
  - stage B streams one head x one 1024-wide sq block at a time: score psum
    tiles double-buffered (2x2 banks) so the PE runs ahead of the exp;
    ctx accumulates in a 3rd/4th bank pair.  Score matmuls for one (h, sk)
    share one LDWEIGHTS (j-inner ordering), same for attn@V.
  - ctx psum is released immediately after a raw bf16 copy to SBUF; the
    softmax division (reciprocal+broadcast+multiply) happens out of the
    critical loop.
  - output projection is emitted interleaved with the following sq-block's
    attention so its matmuls fill PE slack; results staged bf16 and the
    host sums partials in fp32.
  - out/ctx copies and V-wave DMAs sliced so xq/xk/xv are each read once.
"""

import numpy as np
import ml_dtypes

import concourse.bass as bass
import concourse.mybir as mybir
import concourse.tile as tile
from concourse import bacc, library_config
from concourse.bass_utils import run_bass_kernel_spmd

# Problem shapes (hardcoded per contest rules).
B, S, D, H, DH = 2, 2048, 1024, 16, 64
NCORES = 8
NH = 4            # heads per core
DQ = NH * DH      # 256: per-core q/k/v width
P = 128

F32 = mybir.dt.float32
BF16 = mybir.dt.bfloat16
NP_BF16 = ml_dtypes.bfloat16

SQC = 1024        # sq block width (exp grain; 2 psum banks)
FDP = 512         # matmul moving free-dim (one fp32 psum bank)


def build_nc(s=S, d=D):
    """Build the per-core Bass program (same NEFF on all 8 cores)."""
    ko = d // P           # 8 contraction chunks for projections
    mq = DQ // P          # 2 q/k partition chunks
    skn = s // P          # 16 sk chunks
    nsq = s // SQC        # 2 sq blocks
    nsf = s // FDP        # 4 projection n-chunks
    nj = SQC // FDP       # 2 matmuls per score tile

    nc = bacc.Bacc("TRN2", debug=False)

    xq_t = nc.declare_dram_parameter("xq", [d, s], BF16, isOutput=False)
    xk_t = nc.declare_dram_parameter("xk", [d, s], BF16, isOutput=False)
    xv_t = nc.declare_dram_parameter("xv", [d, s], BF16, isOutput=False)
    wq_t = nc.declare_dram_parameter("wq", [d, DQ], BF16, isOutput=False)
    wk_t = nc.declare_dram_parameter("wk", [d, DQ], BF16, isOutput=False)
    wv_t = nc.declare_dram_parameter("wv", [d, DQ], BF16, isOutput=False)
    wo_t = nc.declare_dram_parameter("wo", [DQ, d], BF16, isOutput=False)
    bq_t = nc.declare_dram_parameter("bq", [P, mq], F32, isOutput=False)
    bk_t = nc.declare_dram_parameter("bk", [P, mq], F32, isOutput=False)
    bv_t = nc.declare_dram_parameter("bv", [P, DQ], F32, isOutput=False)
    keep_t = nc.declare_dram_parameter("keep", [s, s], BF16, isOutput=False)
    out_t = nc.declare_dram_parameter("out", [d, s], BF16, isOutput=True)

    AF = mybir.ActivationFunctionType
    OP = mybir.AluOpType

    with tile.TileContext(nc) as tc:
        nc.gpsimd.load_library(library_config.attn)
        with (
            tc.tile_pool(name="const", bufs=1) as const,
            tc.tile_pool(name="xs", bufs=4) as xs,
            tc.tile_pool(name="attn", bufs=4) as attnp,
            tc.tile_pool(name="sc", bufs=2) as scp,
            tc.tile_pool(name="outp", bufs=4) as outp,
            tc.tile_pool(name="pss", bufs=3, space="PSUM") as pss,
            tc.tile_pool(name="psc", bufs=1, space="PSUM") as psc,
        ):
            # ---- persistent SBUF tensors ----
            wq_sb = const.tile([P, ko, DQ], BF16, tag="wq")
            wk_sb = const.tile([P, ko, DQ], BF16, tag="wk")
            wv_sb = const.tile([P, ko, DQ], BF16, tag="wv")
            wo_sb = const.tile([P, mq, d], BF16, tag="wo")
            bq_sb = const.tile([P, mq], F32, tag="bq")
            bk_sb = const.tile([P, mq], F32, tag="bk")
            bv_sb = const.tile([P, DQ], F32, tag="bv")
            qT_sb = const.tile([P, mq, s], BF16, tag="qT")
            kT_sb = const.tile([P, mq, s], BF16, tag="kT")
            v_sb = const.tile([P, skn, NH * 65], BF16, tag="v")
            keep_sb = const.tile([P, skn, s], BF16, tag="keep")
            ctxT_sb = const.tile([P, mq, s], BF16, tag="ctxT")

            nc.sync.dma_start(wq_sb, wq_t[:].rearrange("(ko p) m -> p ko m", p=P))
            nc.sync.dma_start(wk_sb, wk_t[:].rearrange("(ko p) m -> p ko m", p=P))
            nc.sync.dma_start(wv_sb, wv_t[:].rearrange("(ko p) m -> p ko m", p=P))
            nc.sync.dma_start(wo_sb, wo_t[:].rearrange("(mq p) n -> p mq n", p=P))
            nc.sync.dma_start(bq_sb, bq_t[:])
            nc.sync.dma_start(bk_sb, bk_t[:])
            nc.sync.dma_start(bv_sb, bv_t[:])

            # ones column per head in the V tile (softmax denominator trick)
            nc.vector.memset(
                v_sb[:].rearrange("p s (h c) -> p s h c", h=NH)[:, :, :, 64:65], 1.0
            )

            # ---- stage A: projections, kk-outer (dense PE stream) ----
            def project_qk(x_t, w_sb, b_sb, dst_sb):
                # 8 accumulation groups (m, n) of [128, FDP] across 4 tiles
                tiles = [pss.tile([P, SQC], F32, name="pj0", tag="s"),
                         pss.tile([P, SQC], F32, name="pj1", tag="s"),
                         pss.tile([P, SQC], F32, name="pj2", tag="s"),
                         psc.tile([P, SQC], F32, name="pj3", tag="c")]

                def gsl(g):
                    return tiles[g // nj][:, (g % nj) * FDP:(g % nj + 1) * FDP]

                for kk in range(ko):
                    t = xs.tile([P, s], BF16, tag="xt")
                    eng = nc.sync if kk % 2 == 0 else nc.scalar
                    eng.dma_start(t, x_t[kk * P:(kk + 1) * P, :])
                    for m in range(mq):
                        for n in range(nsf):
                            nc.tensor.matmul(
                                gsl(m * nsf + n),
                                w_sb[:, kk, m * P:(m + 1) * P],
                                t[:, n * FDP:(n + 1) * FDP],
                                start=(kk == 0),
                                stop=(kk == ko - 1),
                            )
                for m in range(mq):
                    for n in range(nsf):
                        nc.vector.tensor_scalar_add(
                            dst_sb[:, m, n * FDP:(n + 1) * FDP],
                            gsl(m * nsf + n),
                            b_sb[:, m:m + 1],
                        )

            with nc.named_scope("kproj"):
                project_qk(xk_t, wk_sb, bk_sb, kT_sb)
            with nc.named_scope("qproj"):
                project_qk(xq_t, wq_sb, bq_sb, qT_sb)

            # v projection: v[sv, dv] = sum_d xvT[d, sv] * wvT[d, dv]
            # waves of 4 sv chunks; xv DMA'd in 512-col slices (read once)
            v_strided = v_sb[:].rearrange("p s (h c) -> p s h c", h=NH)
            with nc.named_scope("vproj"):
                for w in range(skn // 4):
                    # 4 sv chunks per wave; groups bank-aligned (FDP grain):
                    # sv i lives in tile i//2 at column (i%2)*FDP, 256 wide.
                    vts = [pss.tile([P, SQC], F32, name="vts", tag="s"),
                           pss.tile([P, SQC], F32, name="vtc", tag="s")]

                    def vsl(i):
                        return vts[i // 2][:, (i % 2) * FDP:(i % 2) * FDP + DQ]

                    for kk in range(ko):
                        t = xs.tile([P, 4 * P], BF16, tag="xv")
                        eng = nc.sync if kk % 2 == 0 else nc.scalar
                        eng.dma_start(
                            t, xv_t[kk * P:(kk + 1) * P,
                                    w * 4 * P:(w + 1) * 4 * P])
                        for i in range(4):
                            nc.tensor.matmul(
                                vsl(i),
                                t[:, i * P:(i + 1) * P],
                                wv_sb[:, kk, :],
                                start=(kk == 0),
                                stop=(kk == ko - 1),
                            )
                    for i in range(4):
                        sv = w * 4 + i
                        nc.vector.tensor_tensor(
                            v_strided[:, sv, :, 0:64],
                            vsl(i).rearrange("p (h c) -> p h c", h=NH),
                            bv_sb[:].rearrange("p (h c) -> p h c", h=NH),
                            OP.add,
                        )

            # keep-mask: [sk partitions, sq free] — after vproj so it does not
            # steal HBM bandwidth from the projection input streams
            for c in range(skn):
                eng = nc.sync if c % 2 == 0 else nc.scalar
                eng.dma_start(keep_sb[:, c, :], keep_t[c * P:(c + 1) * P, :])

            # ---- out-projection emitter (one do-chunk = one psum tile) ----
            def emit_outproj(sqh, do):
                ps = psc.tile([P, SQC], F32, tag="c")
                sq0 = sqh * SQC
                for kk in range(mq):
                    for n in range(nj):
                        nc.tensor.matmul(
                            ps[:, n * FDP:(n + 1) * FDP],
                            wo_sb[:, kk, do * P:(do + 1) * P],
                            ctxT_sb[:, kk, sq0 + n * FDP:sq0 + (n + 1) * FDP],
                            start=(kk == 0),
                            stop=(kk == mq - 1),
                        )
                for n in range(nj):
                    ot = outp.tile([P, FDP], BF16, tag="ot")
                    nc.vector.tensor_copy(ot, ps[:, n * FDP:(n + 1) * FDP])
                    nc.sync.dma_start(
                        out_t[do * P:(do + 1) * P,
                              sq0 + n * FDP:sq0 + (n + 1) * FDP],
                        ot,
                    )

            # ---- stage B: attention, one (sq block, head) per ctx psum ----
            for blk in range(nsq * NH):
                sqh, h = divmod(blk, NH)
                sq0 = sqh * SQC
                hb, hm = (h % 2) * 64, h // 2
                with nc.named_scope(f"attn{blk}"):
                    cps = psc.tile([P, SQC], F32, name="cps", tag="c")[:65, :]
                    for sk in range(skn):
                        sps = pss.tile([P, SQC], F32, tag="s")
                        for j in range(nj):
                            nc.tensor.matmul(
                                sps[:, j * FDP:(j + 1) * FDP],
                                kT_sb[hb:hb + 64, hm, sk * P:(sk + 1) * P],
                                qT_sb[hb:hb + 64, hm,
                                      sq0 + j * FDP:sq0 + (j + 1) * FDP],
                                start=True,
                                stop=True,
                            )
                        at = attnp.tile([P, SQC], BF16, tag="at")
                        nc.scalar.activation(at, sps, AF.Exp, scale=0.125)
                        nc.vector.tensor_tensor(
                            at, at, keep_sb[:, sk, sq0:sq0 + SQC], OP.mult)
                        for j in range(nj):
                            nc.tensor.matmul(
                                cps[:, j * FDP:(j + 1) * FDP],
                                v_sb[:, sk, h * 65:(h + 1) * 65],
                                at[:, j * FDP:(j + 1) * FDP],
                                start=(sk == 0),
                                stop=(sk == skn - 1),
                            )
                    # raw ctx + den out of psum fast; normalize off-critical
                    aug = scp.tile([64, SQC], BF16, tag="aug")
                    nc.vector.tensor_copy(aug, cps[0:64, :])
                    dsb = scp.tile([65, SQC], F32, tag="dsb")
                    nc.vector.tensor_copy(dsb[64:65, :], cps[64:65, :])
                    den0 = scp.tile([1, SQC], F32, tag="den0")
                    nc.sync.dma_start(den0, dsb[64:65, :])
                    nc.vector.reciprocal_approx_fast(out=den0, in_=den0)
                    scl = scp.tile([64, SQC], F32, tag="scl")
                    nc.gpsimd.partition_broadcast(scl, den0[0:1, :])
                    cn = scp.tile([64, SQC], BF16, tag="cn")
                    nc.vector.tensor_tensor(cn, aug, scl, OP.mult)
                    nc.sync.dma_start(
                        ctxT_sb[hb:hb + 64, hm, sq0:sq0 + SQC], cn)
                # interleave previous sq block's output projection
                if sqh == nsq - 1:
                    with nc.named_scope("oproj0"):
                        for do in range(h * ko // NH, (h + 1) * ko // NH):
                            emit_outproj(0, do)
            with nc.named_scope("oproj1"):
                for do in range(ko):
                    emit_outproj(1, do)
    nc.compile()
    return nc


_NC_CACHE = {}


def _get_nc(s=S, d=D):
    key = (s, d, SQC)
    if key not in _NC_CACHE:
        _NC_CACHE[key] = build_nc(s, d)
    return _NC_CACHE[key]


def make_in_maps(query, key, value, mask, Wq, bq, Wk, bk, Wv, bv, Wo, bo,
                 s=S, d=D):
    """Build the 8 per-core input maps (host-side shard + layout prep)."""
    nb = query.shape[0]
    per_b = []
    for b in range(nb):
        xqT = np.ascontiguousarray(query[b].T).astype(NP_BF16)
        xkT = np.ascontiguousarray(key[b].T).astype(NP_BF16)
        xvT = np.ascontiguousarray(value[b].T).astype(NP_BF16)
        keepT = np.ascontiguousarray((~mask[b, 0]).T).astype(NP_BF16)
        per_b.append((xqT, xkT, xvT, keepT))
    per_g = []
    for g in range(4):
        sl = slice(g * DQ, (g + 1) * DQ)
        per_g.append((
            np.ascontiguousarray(Wq[sl].T).astype(NP_BF16),
            np.ascontiguousarray(Wk[sl].T).astype(NP_BF16),
            np.ascontiguousarray(Wv[sl].T).astype(NP_BF16),
            np.ascontiguousarray(Wo[:, sl].T).astype(NP_BF16),
            np.ascontiguousarray(bq[sl].reshape(DQ // P, P).T).astype(np.float32),
            np.ascontiguousarray(bk[sl].reshape(DQ // P, P).T).astype(np.float32),
            np.ascontiguousarray(np.broadcast_to(bv[sl], (P, DQ))).astype(np.float32),
        ))
    in_maps = []
    for c in range(NCORES):
        b, g = c // 4, c % 4
        xqT, xkT, xvT, keepT = per_b[b % nb]
        wqT, wkT, wvT, woT, bq2, bk2, bvr = per_g[g]
        in_maps.append({
            "xq": xqT, "xk": xkT, "xv": xvT,
            "wq": wqT, "wk": wkT, "wv": wvT, "wo": woT,
            "bq": bq2, "bk": bk2, "bv": bvr,
            "keep": keepT,
        })
    return in_maps


def gather_output(results, bo, nb=B, s=S, d=D):
    out = np.empty((nb, s, d), np.float32)
    for b in range(nb):
        acc = results[4 * b]["out"].astype(np.float32)
        for g in range(1, 4):
            acc += results[4 * b + g]["out"].astype(np.float32)
        out[b] = acc.T
    out += bo.astype(np.float32)
    return out


def run_on_cores(in_maps, trace=False, **kw):
    nc = _get_nc()
    return run_bass_kernel_spmd(nc, in_maps, list(range(NCORES)), trace=trace, **kw)


def kernel(query, key, value, mask, Wq, bq, Wk, bk, Wv, bv, Wo, bo):
    in_maps = make_in_maps(query, key, value, mask,
                           Wq, bq, Wk, bk, Wv, bv, Wo, bo)
    res = run_on_cores(in_maps, trace=False)
    return gather_output(res.results, bo)


# revision 19
# speedup vs baseline: 1.0419x; 1.0419x over previous
"""Multi-head attention Bass kernel for Trainium2, sharded over 8 NeuronCores.

Sharding: core c handles batch b = c//4 and head-group g = c%4 (4 of 16 heads,
i.e. a 256-wide slice of the QKV projection output).  Each core computes its
heads' attention and a partial output projection (contribution of its 256
ctx columns to the full [S, D] output).  The host sums the 4 partials per
batch and adds the output bias.

Device-side layout choices:
  - activations shipped pre-transposed: xT = x.T  [D, S] so the contraction
    dim (D) lands on SBUF partitions without any on-device transpose.
  - scores are computed transposed (scoresT[sk, sq]) so the attention weights
    leave softmax with sk on partitions — the contraction layout attn@V needs.
  - softmax denominator comes free from a ones-column appended to V
    (ctx psum row 64 = sum_sk attn);  no max-subtraction (scores bounded).
  - masking is a multiply by a 0/1 bf16 keep-mask after exp.

v2 pipeline (vs the 368us baseline whose PE ran cold and stalled on psum):
  - stage B streams one head x one 1024-wide sq block at a time: score psum
    tiles double-buffered (2x2 banks) so the PE runs ahead of the exp;
    ctx accumulates in a 3rd/4th bank pair.  Score matmuls for one (h, sk)
    share one LDWEIGHTS (j-inner ordering), same for attn@V.
  - ctx psum is released immediately after a raw bf16 copy to SBUF; the
    softmax division (reciprocal+broadcast+multiply) happens out of the
    critical loop.
  - output projection is emitted interleaved with the following sq-block's
    attention so its matmuls fill PE slack; results staged bf16 and the
    host sums partials in fp32.
  - out/ctx copies and V-wave DMAs sliced so xq/xk/xv are each read once.
"""

import numpy as np
import ml_dtypes

import concourse.bass as bass
import concourse.mybir as mybir
import concourse.tile as tile
from concourse import bacc, library_config
from concourse.bass_utils import run_bass_kernel_spmd

# Problem shapes (hardcoded per contest rules).
B, S, D, H, DH = 2, 2048, 1024, 16, 64
NCORES = 8
NH = 4            # heads per core
DQ = NH * DH      # 256: per-core q/k/v width
P = 128

F32 = mybir.dt.float32
BF16 = mybir.dt.bfloat16
NP_BF16 = ml_dtypes.bfloat16

SQC = 1024        # sq block width (exp grain; 2 psum banks)
FDP = 512         # matmul moving free-dim (one fp32 psum bank)


def build_nc(s=S, d=D):
    """Build the per-core Bass program (same NEFF on all 8 cores)."""
    ko = d // P           # 8 contraction chunks for projections
    mq = DQ // P          # 2 q/k partition chunks
    skn = s // P          # 16 sk chunks
    nsq = s // SQC        # 2 sq blocks
    nsf = s // FDP        # 4 projection n-chunks
    nj = SQC // FDP       # 2 matmuls per score tile

    nc = bacc.Bacc("TRN2", debug=False)

    xq_t = nc.declare_dram_parameter("xq", [d, s], BF16, isOutput=False)
    xk_t = nc.declare_dram_parameter("xk", [d, s], BF16, isOutput=False)
    xv_t = nc.declare_dram_parameter("xv", [d, s], BF16, isOutput=False)
    wq_t = nc.declare_dram_parameter("wq", [d, DQ], BF16, isOutput=False)
    wk_t = nc.declare_dram_parameter("wk", [d, DQ], BF16, isOutput=False)
    wv_t = nc.declare_dram_parameter("wv", [d, DQ], BF16, isOutput=False)
    wo_t = nc.declare_dram_parameter("wo", [DQ, d], BF16, isOutput=False)
    bq_t = nc.declare_dram_parameter("bq", [P, mq], F32, isOutput=False)
    bk_t = nc.declare_dram_parameter("bk", [P, mq], F32, isOutput=False)
    bv_t = nc.declare_dram_parameter("bv", [P, DQ], F32, isOutput=False)
    keep_t = nc.declare_dram_parameter("keep", [s, s], BF16, isOutput=False)
    out_t = nc.declare_dram_parameter("out", [d, s], BF16, isOutput=True)

    AF = mybir.ActivationFunctionType
    OP = mybir.AluOpType

    with tile.TileContext(nc) as tc:
        nc.gpsimd.load_library(library_config.attn)
        with (
            tc.tile_pool(name="const", bufs=1) as const,
            tc.tile_pool(name="xs", bufs=6) as xs,
            tc.tile_pool(name="attn", bufs=4) as attnp,
            tc.tile_pool(name="sc", bufs=2) as scp,
            tc.tile_pool(name="outp", bufs=4) as outp,
            tc.tile_pool(name="pss", bufs=2, space="PSUM") as pss,
            tc.tile_pool(name="psc", bufs=1, space="PSUM") as psc,
            tc.tile_pool(name="pso", bufs=2, space="PSUM") as pso,
        ):
            # ---- persistent SBUF tensors ----
            wq_sb = const.tile([P, ko, DQ], BF16, tag="wq")
            wk_sb = const.tile([P, ko, DQ], BF16, tag="wk")
            wv_sb = const.tile([P, ko, DQ], BF16, tag="wv")
            wo_sb = const.tile([P, mq, d], BF16, tag="wo")
            bq_sb = const.tile([P, mq], F32, tag="bq")
            bk_sb = const.tile([P, mq], F32, tag="bk")
            bv_sb = const.tile([P, DQ], F32, tag="bv")
            qT_sb = const.tile([P, mq, s], BF16, tag="qT")
            kT_sb = const.tile([P, mq, s], BF16, tag="kT")
            v_sb = const.tile([P, skn, NH * 65], BF16, tag="v")
            keep_sb = const.tile([P, skn, s], BF16, tag="keep")
            ctxT_sb = const.tile([P, mq, s], BF16, tag="ctxT")

            nc.sync.dma_start(wq_sb, wq_t[:].rearrange("(ko p) m -> p ko m", p=P))
            nc.sync.dma_start(wk_sb, wk_t[:].rearrange("(ko p) m -> p ko m", p=P))
            nc.sync.dma_start(wv_sb, wv_t[:].rearrange("(ko p) m -> p ko m", p=P))
            nc.sync.dma_start(wo_sb, wo_t[:].rearrange("(mq p) n -> p mq n", p=P))
            nc.sync.dma_start(bq_sb, bq_t[:])
            nc.sync.dma_start(bk_sb, bk_t[:])
            nc.sync.dma_start(bv_sb, bv_t[:])

            # ones column per head in the V tile (softmax denominator trick)
            nc.vector.memset(
                v_sb[:].rearrange("p s (h c) -> p s h c", h=NH)[:, :, :, 64:65], 1.0
            )

            # ---- stage A: projections, kk-outer (dense PE stream) ----
            def project_qk(x_t, w_sb, b_sb, dst_sb):
                # 8 accumulation groups (m, n) of [128, FDP] across 4 tiles
                tiles = [pss.tile([P, SQC], F32, name="pj0", tag="s"),
                         pss.tile([P, SQC], F32, name="pj1", tag="s"),
                         psc.tile([P, SQC], F32, name="pj2", tag="c"),
                         pso.tile([P, FDP], F32, name="pj3", tag="o"),
                         pso.tile([P, FDP], F32, name="pj4", tag="o")]

                def gsl(g):
                    if g < 4:
                        return tiles[g // nj][:, (g % nj) * FDP:(g % nj + 1) * FDP]
                    if g < 6:
                        return tiles[2][:, (g - 4) * FDP:(g - 3) * FDP]
                    return tiles[g - 3]

                for kk in range(ko):
                    t = xs.tile([P, s], BF16, tag="xt")
                    eng = nc.sync if kk % 2 == 0 else nc.scalar
                    eng.dma_start(t, x_t[kk * P:(kk + 1) * P, :])
                    for m in range(mq):
                        for n in range(nsf):
                            nc.tensor.matmul(
                                gsl(m * nsf + n),
                                w_sb[:, kk, m * P:(m + 1) * P],
                                t[:, n * FDP:(n + 1) * FDP],
                                start=(kk == 0),
                                stop=(kk == ko - 1),
                            )
                for m in range(mq):
                    for n in range(nsf):
                        nc.vector.tensor_scalar_add(
                            dst_sb[:, m, n * FDP:(n + 1) * FDP],
                            gsl(m * nsf + n),
                            b_sb[:, m:m + 1],
                        )

            with nc.named_scope("kproj"):
                project_qk(xk_t, wk_sb, bk_sb, kT_sb)
            with nc.named_scope("qproj"):
                project_qk(xq_t, wq_sb, bq_sb, qT_sb)

            # v projection: v[sv, dv] = sum_d xvT[d, sv] * wvT[d, dv]
            # waves of 4 sv chunks; xv DMA'd in 512-col slices (read once)
            v_strided = v_sb[:].rearrange("p s (h c) -> p s h c", h=NH)
            with nc.named_scope("vproj"):
                for w in range(skn // 4):
                    # 4 sv chunks per wave; groups bank-aligned (FDP grain):
                    # sv i lives in tile i//2 at column (i%2)*FDP, 256 wide.
                    vts = [pss.tile([P, SQC], F32, name="vts", tag="s"),
                           pss.tile([P, SQC], F32, name="vtc", tag="s")]

                    def vsl(i):
                        return vts[i // 2][:, (i % 2) * FDP:(i % 2) * FDP + DQ]

                    for kk in range(ko):
                        t = xs.tile([P, 4 * P], BF16, tag="xv")
                        eng = nc.sync if kk % 2 == 0 else nc.scalar
                        eng.dma_start(
                            t, xv_t[kk * P:(kk + 1) * P,
                                    w * 4 * P:(w + 1) * 4 * P])
                        for i in range(4):
                            nc.tensor.matmul(
                                vsl(i),
                                t[:, i * P:(i + 1) * P],
                                wv_sb[:, kk, :],
                                start=(kk == 0),
                                stop=(kk == ko - 1),
                            )
                    for i in range(4):
                        sv = w * 4 + i
                        nc.vector.tensor_tensor(
                            v_strided[:, sv, :, 0:64],
                            vsl(i).rearrange("p (h c) -> p h c", h=NH),
                            bv_sb[:].rearrange("p (h c) -> p h c", h=NH),
                            OP.add,
                        )

            # keep-mask: [sk partitions, sq free] — after vproj so it does not
            # steal HBM bandwidth from the projection input streams
            for c in range(skn):
                eng = nc.sync if c % 2 == 0 else nc.scalar
                eng.dma_start(keep_sb[:, c, :], keep_t[c * P:(c + 1) * P, :])

            # ---- out-projection emitter (one do-chunk = one psum tile) ----
            def emit_outproj(sqh, do):
                sq0 = sqh * SQC
                pts = [pso.tile([P, FDP], F32, name="op0", tag="o"),
                       pso.tile([P, FDP], F32, name="op1", tag="o")]
                for kk in range(mq):
                    for n in range(nj):
                        nc.tensor.matmul(
                            pts[n],
                            wo_sb[:, kk, do * P:(do + 1) * P],
                            ctxT_sb[:, kk, sq0 + n * FDP:sq0 + (n + 1) * FDP],
                            start=(kk == 0),
                            stop=(kk == mq - 1),
                        )
                for n in range(nj):
                    ot = outp.tile([P, FDP], BF16, tag="ot")
                    nc.vector.tensor_copy(ot, pts[n])
                    nc.sync.dma_start(
                        out_t[do * P:(do + 1) * P,
                              sq0 + n * FDP:sq0 + (n + 1) * FDP],
                        ot,
                    )

            # ---- stage B: attention, one (sq block, head) per ctx psum ----
            for blk in range(nsq * NH):
                sqh, h = divmod(blk, NH)
                sq0 = sqh * SQC
                hb, hm = (h % 2) * 64, h // 2
                with nc.named_scope(f"attn{blk}"):
                    cps = psc.tile([P, SQC], F32, name="cps", tag="c")[:65, :]
                    for sk in range(skn):
                        sps = pss.tile([P, SQC], F32, tag="s")
                        for j in range(nj):
                            nc.tensor.matmul(
                                sps[:, j * FDP:(j + 1) * FDP],
                                kT_sb[hb:hb + 64, hm, sk * P:(sk + 1) * P],
                                qT_sb[hb:hb + 64, hm,
                                      sq0 + j * FDP:sq0 + (j + 1) * FDP],
                                start=True,
                                stop=True,
                            )
                        at = attnp.tile([P, SQC], BF16, tag="at")
                        nc.scalar.activation(at, sps, AF.Exp, scale=0.125)
                        nc.vector.tensor_tensor(
                            at, at, keep_sb[:, sk, sq0:sq0 + SQC], OP.mult)
                        for j in range(nj):
                            nc.tensor.matmul(
                                cps[:, j * FDP:(j + 1) * FDP],
                                v_sb[:, sk, h * 65:(h + 1) * 65],
                                at[:, j * FDP:(j + 1) * FDP],
                                start=(sk == 0),
                                stop=(sk == skn - 1),
                            )
                    # raw ctx + den out of psum fast; normalize off-critical
                    aug = scp.tile([64, SQC], BF16, tag="aug")
                    nc.vector.tensor_copy(aug, cps[0:64, :])
                    dsb = scp.tile([65, SQC], F32, tag="dsb")
                    nc.vector.tensor_copy(dsb[64:65, :], cps[64:65, :])
                    den0 = scp.tile([1, SQC], F32, tag="den0")
                    nc.sync.dma_start(den0, dsb[64:65, :])
                    nc.vector.reciprocal_approx_fast(out=den0, in_=den0)
                    scl = scp.tile([64, SQC], F32, tag="scl")
                    nc.gpsimd.partition_broadcast(scl, den0[0:1, :])
                    cn = scp.tile([64, SQC], BF16, tag="cn")
                    nc.vector.tensor_tensor(cn, aug, scl, OP.mult)
                    nc.sync.dma_start(
                        ctxT_sb[hb:hb + 64, hm, sq0:sq0 + SQC], cn)
                # interleave previous sq block's output projection
                if sqh == nsq - 1:
                    with nc.named_scope("oproj0"):
                        for do in range(h * ko // NH, (h + 1) * ko // NH):
                            emit_outproj(0, do)
            with nc.named_scope("oproj1"):
                for do in range(ko):
                    emit_outproj(1, do)
    nc.compile()
    return nc


_NC_CACHE = {}


def _get_nc(s=S, d=D):
    key = (s, d, SQC)
    if key not in _NC_CACHE:
        _NC_CACHE[key] = build_nc(s, d)
    return _NC_CACHE[key]


def make_in_maps(query, key, value, mask, Wq, bq, Wk, bk, Wv, bv, Wo, bo,
                 s=S, d=D):
    """Build the 8 per-core input maps (host-side shard + layout prep)."""
    nb = query.shape[0]
    per_b = []
    for b in range(nb):
        xqT = np.ascontiguousarray(query[b].T).astype(NP_BF16)
        xkT = np.ascontiguousarray(key[b].T).astype(NP_BF16)
        xvT = np.ascontiguousarray(value[b].T).astype(NP_BF16)
        keepT = np.ascontiguousarray((~mask[b, 0]).T).astype(NP_BF16)
        per_b.append((xqT, xkT, xvT, keepT))
    per_g = []
    for g in range(4):
        sl = slice(g * DQ, (g + 1) * DQ)
        per_g.append((
            np.ascontiguousarray(Wq[sl].T).astype(NP_BF16),
            np.ascontiguousarray(Wk[sl].T).astype(NP_BF16),
            np.ascontiguousarray(Wv[sl].T).astype(NP_BF16),
            np.ascontiguousarray(Wo[:, sl].T).astype(NP_BF16),
            np.ascontiguousarray(bq[sl].reshape(DQ // P, P).T).astype(np.float32),
            np.ascontiguousarray(bk[sl].reshape(DQ // P, P).T).astype(np.float32),
            np.ascontiguousarray(np.broadcast_to(bv[sl], (P, DQ))).astype(np.float32),
        ))
    in_maps = []
    for c in range(NCORES):
        b, g = c // 4, c % 4
        xqT, xkT, xvT, keepT = per_b[b % nb]
        wqT, wkT, wvT, woT, bq2, bk2, bvr = per_g[g]
        in_maps.append({
            "xq": xqT, "xk": xkT, "xv": xvT,
            "wq": wqT, "wk": wkT, "wv": wvT, "wo": woT,
            "bq": bq2, "bk": bk2, "bv": bvr,
            "keep": keepT,
        })
    return in_maps


def gather_output(results, bo, nb=B, s=S, d=D):
    out = np.empty((nb, s, d), np.float32)
    for b in range(nb):
        acc = results[4 * b]["out"].astype(np.float32)
        for g in range(1, 4):
            acc += results[4 * b + g]["out"].astype(np.float32)
        out[b] = acc.T
    out += bo.astype(np.float32)
    return out


def run_on_cores(in_maps, trace=False, **kw):
    nc = _get_nc()
    return run_bass_kernel_spmd(nc, in_maps, list(range(NCORES)), trace=trace, **kw)


def kernel(query, key, value, mask, Wq, bq, Wk, bk, Wv, bv, Wo, bo):
    in_maps = make_in_maps(query, key, value, mask,
                           Wq, bq, Wk, bk, Wv, bv, Wo, bo)
    res = run_on_cores(in_maps, trace=False)
    return gather_output(res.results, bo)


# revision 23
# speedup vs baseline: 1.1202x; 1.0752x over previous
"""Multi-head attention Bass kernel for Trainium2, sharded over 8 NeuronCores.

Sharding: core c handles batch b = c//4 and head-group g = c%4 (4 of 16 heads,
i.e. a 256-wide slice of the QKV projection output).  Each core computes its
heads' attention and a partial output projection (contribution of its 256
ctx columns to the full [S, D] output).  The host sums the 4 partials per
batch and adds the output bias.

Device-side layout choices:
  - activations shipped pre-transposed: xT = x.T  [D, S] so the contraction
    dim (D) lands on SBUF partitions without any on-device transpose.
  - scores are computed transposed (scoresT[sk, sq]) so the attention weights
    leave softmax with sk on partitions — the contraction layout attn@V needs.
  - softmax denominator comes free from a ones-column appended to V
    (ctx psum row 64 = sum_sk attn);  no max-subtraction (scores bounded).
  - masking is a multiply by a 0/1 bf16 keep-mask after exp.

v2 pipeline (vs the 368us baseline whose PE ran cold and stalled on psum):
  - stage B streams one head x one 1024-wide sq block at a time: score psum
    tiles double-buffered (2x2 banks) so the PE runs ahead of the exp;
    ctx accumulates in a 3rd/4th bank pair.  Score matmuls for one (h, sk)
    share one LDWEIGHTS (j-inner ordering), same for attn@V.
  - ctx psum is released immediately after a raw bf16 copy to SBUF; the
    softmax division (reciprocal+broadcast+multiply) happens out of the
    critical loop.
  - output projection is emitted interleaved with the following sq-block's
    attention so its matmuls fill PE slack; results staged bf16 and the
    host sums partials in fp32.
  - out/ctx copies and V-wave DMAs sliced so xq/xk/xv are each read once.
"""

import numpy as np
import ml_dtypes

import concourse.bass as bass
import concourse.mybir as mybir
import concourse.tile as tile
from concourse import bacc, library_config
from concourse.bass_utils import run_bass_kernel_spmd

# Problem shapes (hardcoded per contest rules).
B, S, D, H, DH = 2, 2048, 1024, 16, 64
NCORES = 8
NH = 4            # heads per core
DQ = NH * DH      # 256: per-core q/k/v width
P = 128

F32 = mybir.dt.float32
BF16 = mybir.dt.bfloat16
NP_BF16 = ml_dtypes.bfloat16

SQC = 1024        # sq block width (exp grain; 2 psum banks)
FDP = 512         # matmul moving free-dim (one fp32 psum bank)


def build_nc(s=S, d=D):
    """Build the per-core Bass program (same NEFF on all 8 cores)."""
    ko = d // P           # 8 contraction chunks for projections
    mq = DQ // P          # 2 q/k partition chunks
    skn = s // P          # 16 sk chunks
    nsq = s // SQC        # 2 sq blocks
    nsf = s // FDP        # 4 projection n-chunks
    nj = SQC // FDP       # 2 matmuls per score tile

    nc = bacc.Bacc("TRN2", debug=False)

    xq_t = nc.declare_dram_parameter("xq", [d, s], BF16, isOutput=False)
    xk_t = nc.declare_dram_parameter("xk", [d, s], BF16, isOutput=False)
    xv_t = nc.declare_dram_parameter("xv", [d, s], BF16, isOutput=False)
    wq_t = nc.declare_dram_parameter("wq", [d, DQ], BF16, isOutput=False)
    wk_t = nc.declare_dram_parameter("wk", [d, DQ], BF16, isOutput=False)
    wv_t = nc.declare_dram_parameter("wv", [d, DQ], BF16, isOutput=False)
    wo_t = nc.declare_dram_parameter("wo", [DQ, d], BF16, isOutput=False)
    bq_t = nc.declare_dram_parameter("bq", [P, mq], F32, isOutput=False)
    bk_t = nc.declare_dram_parameter("bk", [P, mq], F32, isOutput=False)
    bv_t = nc.declare_dram_parameter("bv", [P, DQ], F32, isOutput=False)
    keep_t = nc.declare_dram_parameter("keep", [s, s], BF16, isOutput=False)
    out_t = nc.declare_dram_parameter("out", [d, s], BF16, isOutput=True)

    AF = mybir.ActivationFunctionType
    OP = mybir.AluOpType

    with tile.TileContext(nc) as tc:
        nc.gpsimd.load_library(library_config.attn)
        with (
            tc.tile_pool(name="const", bufs=1) as const,
            tc.tile_pool(name="xs", bufs=6) as xs,
            tc.tile_pool(name="attn", bufs=4) as attnp,
            tc.tile_pool(name="sc", bufs=2) as scp,
            tc.tile_pool(name="outp", bufs=4) as outp,
            tc.tile_pool(name="pss", bufs=2, space="PSUM") as pss,
            tc.tile_pool(name="psc", bufs=1, space="PSUM") as psc,
            tc.tile_pool(name="pso", bufs=2, space="PSUM") as pso,
        ):
            # ---- persistent SBUF tensors ----
            wq_sb = const.tile([P, ko, DQ], BF16, tag="wq")
            wk_sb = const.tile([P, ko, DQ], BF16, tag="wk")
            wv_sb = const.tile([P, ko, DQ], BF16, tag="wv")
            wo_sb = const.tile([P, mq, d], BF16, tag="wo")
            bq_sb = const.tile([P, mq], F32, tag="bq")
            bk_sb = const.tile([P, mq], F32, tag="bk")
            bv_sb = const.tile([P, DQ], F32, tag="bv")
            qT_sb = const.tile([P, mq, s], BF16, tag="qT")
            kT_sb = const.tile([P, mq, s], BF16, tag="kT")
            v_sb = const.tile([P, skn, NH * 65], BF16, tag="v")
            keep_sb = const.tile([P, skn, s], BF16, tag="keep")
            ctxT_sb = const.tile([P, mq, s], BF16, tag="ctxT")

            nc.sync.dma_start(wq_sb, wq_t[:].rearrange("(ko p) m -> p ko m", p=P))
            nc.sync.dma_start(wk_sb, wk_t[:].rearrange("(ko p) m -> p ko m", p=P))
            nc.sync.dma_start(wv_sb, wv_t[:].rearrange("(ko p) m -> p ko m", p=P))
            nc.sync.dma_start(wo_sb, wo_t[:].rearrange("(mq p) n -> p mq n", p=P))
            nc.sync.dma_start(bq_sb, bq_t[:])
            nc.sync.dma_start(bk_sb, bk_t[:])
            nc.sync.dma_start(bv_sb, bv_t[:])

            # ones column per head in the V tile (softmax denominator trick)
            nc.vector.memset(
                v_sb[:].rearrange("p s (h c) -> p s h c", h=NH)[:, :, :, 64:65], 1.0
            )

            # ---- stage A: projections, kk-outer (dense PE stream) ----
            def project_qk(x_t, w_sb, b_sb, dst_sb):
                # 8 accumulation groups (m, n) of [128, FDP] across 4 tiles
                tiles = [pss.tile([P, SQC], F32, name="pj0", tag="s"),
                         pss.tile([P, SQC], F32, name="pj1", tag="s"),
                         psc.tile([P, SQC], F32, name="pj2", tag="c"),
                         pso.tile([P, FDP], F32, name="pj3", tag="o"),
                         pso.tile([P, FDP], F32, name="pj4", tag="o")]

                def gsl(g):
                    if g < 4:
                        return tiles[g // nj][:, (g % nj) * FDP:(g % nj + 1) * FDP]
                    if g < 6:
                        return tiles[2][:, (g - 4) * FDP:(g - 3) * FDP]
                    return tiles[g - 3]

                for kk in range(ko):
                    t = xs.tile([P, s], BF16, tag="xt")
                    nc.sync.dma_start(t, x_t[kk * P:(kk + 1) * P, :])
                    for m in range(mq):
                        for n in range(nsf):
                            nc.tensor.matmul(
                                gsl(m * nsf + n),
                                w_sb[:, kk, m * P:(m + 1) * P],
                                t[:, n * FDP:(n + 1) * FDP],
                                start=(kk == 0),
                                stop=(kk == ko - 1),
                            )
                for m in range(mq):
                    for n in range(nsf):
                        nc.vector.tensor_scalar_add(
                            dst_sb[:, m, n * FDP:(n + 1) * FDP],
                            gsl(m * nsf + n),
                            b_sb[:, m:m + 1],
                        )

            with nc.named_scope("kproj"):
                project_qk(xk_t, wk_sb, bk_sb, kT_sb)
            # keep-mask streams on the scalar HWDGE queue, in parallel with
            # the x streams on sync; block 0 consumes chunks in sk order.
            for c in range(skn):
                nc.scalar.dma_start(keep_sb[:, c, :], keep_t[c * P:(c + 1) * P, :])
            with nc.named_scope("qproj"):
                project_qk(xq_t, wq_sb, bq_sb, qT_sb)

            # v projection: v[sv, dv] = sum_d xvT[d, sv] * wvT[d, dv]
            # waves of 4 sv chunks; xv DMA'd in 512-col slices (read once)
            v_strided = v_sb[:].rearrange("p s (h c) -> p s h c", h=NH)
            with nc.named_scope("vproj"):
                for w in range(skn // 4):
                    # 4 sv chunks per wave; groups bank-aligned (FDP grain):
                    # sv i lives in tile i//2 at column (i%2)*FDP, 256 wide.
                    vts = [pss.tile([P, SQC], F32, name="vts", tag="s"),
                           pss.tile([P, SQC], F32, name="vtc", tag="s")]

                    def vsl(i):
                        return vts[i // 2][:, (i % 2) * FDP:(i % 2) * FDP + DQ]

                    for kk in range(ko):
                        t = xs.tile([P, 4 * P], BF16, tag="xv")
                        nc.sync.dma_start(
                            t, xv_t[kk * P:(kk + 1) * P,
                                    w * 4 * P:(w + 1) * 4 * P])
                        for i in range(4):
                            nc.tensor.matmul(
                                vsl(i),
                                t[:, i * P:(i + 1) * P],
                                wv_sb[:, kk, :],
                                start=(kk == 0),
                                stop=(kk == ko - 1),
                            )
                    for i in range(4):
                        sv = w * 4 + i
                        nc.vector.tensor_tensor(
                            v_strided[:, sv, :, 0:64],
                            vsl(i).rearrange("p (h c) -> p h c", h=NH),
                            bv_sb[:].rearrange("p (h c) -> p h c", h=NH),
                            OP.add,
                        )

            # ---- out-projection emitter (one do-chunk = one psum tile) ----
            def emit_outproj(sqh, do):
                sq0 = sqh * SQC
                pts = [pso.tile([P, FDP], F32, name="op0", tag="o"),
                       pso.tile([P, FDP], F32, name="op1", tag="o")]
                for kk in range(mq):
                    for n in range(nj):
                        nc.tensor.matmul(
                            pts[n],
                            wo_sb[:, kk, do * P:(do + 1) * P],
                            ctxT_sb[:, kk, sq0 + n * FDP:sq0 + (n + 1) * FDP],
                            start=(kk == 0),
                            stop=(kk == mq - 1),
                        )
                for n in range(nj):
                    ot = outp.tile([P, FDP], BF16, tag="ot")
                    nc.vector.tensor_copy(ot, pts[n])
                    nc.sync.dma_start(
                        out_t[do * P:(do + 1) * P,
                              sq0 + n * FDP:sq0 + (n + 1) * FDP],
                        ot,
                    )

            # ---- stage B: attention, one (sq block, head) per ctx psum ----
            for blk in range(nsq * NH):
                sqh, h = divmod(blk, NH)
                sq0 = sqh * SQC
                hb, hm = (h % 2) * 64, h // 2
                with nc.named_scope(f"attn{blk}"):
                    cps = psc.tile([P, SQC], F32, name="cps", tag="c")[:65, :]
                    for sk in range(skn):
                        sps = pss.tile([P, SQC], F32, tag="s")
                        for j in range(nj):
                            nc.tensor.matmul(
                                sps[:, j * FDP:(j + 1) * FDP],
                                kT_sb[hb:hb + 64, hm, sk * P:(sk + 1) * P],
                                qT_sb[hb:hb + 64, hm,
                                      sq0 + j * FDP:sq0 + (j + 1) * FDP],
                                start=True,
                                stop=True,
                            )
                        at = attnp.tile([P, SQC], BF16, tag="at")
                        nc.scalar.activation(at, sps, AF.Exp, scale=0.125)
                        nc.vector.tensor_tensor(
                            at, at, keep_sb[:, sk, sq0:sq0 + SQC], OP.mult)
                        for j in range(nj):
                            nc.tensor.matmul(
                                cps[:, j * FDP:(j + 1) * FDP],
                                v_sb[:, sk, h * 65:(h + 1) * 65],
                                at[:, j * FDP:(j + 1) * FDP],
                                start=(sk == 0),
                                stop=(sk == skn - 1),
                            )
                    # raw ctx + den out of psum fast; normalize off-critical
                    aug = scp.tile([64, SQC], BF16, tag="aug")
                    nc.vector.tensor_copy(aug, cps[0:64, :])
                    dsb = scp.tile([65, SQC], F32, tag="dsb")
                    nc.vector.tensor_copy(dsb[64:65, :], cps[64:65, :])
                    den0 = scp.tile([1, SQC], F32, tag="den0")
                    nc.sync.dma_start(den0, dsb[64:65, :])
                    nc.vector.reciprocal_approx_fast(out=den0, in_=den0)
                    scl = scp.tile([64, SQC], F32, tag="scl")
                    nc.gpsimd.partition_broadcast(scl, den0[0:1, :])
                    cn = scp.tile([64, SQC], BF16, tag="cn")
                    nc.vector.tensor_tensor(cn, aug, scl, OP.mult)
                    nc.sync.dma_start(
                        ctxT_sb[hb:hb + 64, hm, sq0:sq0 + SQC], cn)
                # interleave previous sq block's output projection
                if sqh == nsq - 1:
                    with nc.named_scope("oproj0"):
                        for do in range(h * ko // NH, (h + 1) * ko // NH):
                            emit_outproj(0, do)
            with nc.named_scope("oproj1"):
                for do in range(ko):
                    emit_outproj(1, do)
    nc.compile()
    return nc


_NC_CACHE = {}


def _get_nc(s=S, d=D):
    key = (s, d, SQC)
    if key not in _NC_CACHE:
        _NC_CACHE[key] = build_nc(s, d)
    return _NC_CACHE[key]


def make_in_maps(query, key, value, mask, Wq, bq, Wk, bk, Wv, bv, Wo, bo,
                 s=S, d=D):
    """Build the 8 per-core input maps (host-side shard + layout prep)."""
    nb = query.shape[0]
    per_b = []
    for b in range(nb):
        xqT = np.ascontiguousarray(query[b].T).astype(NP_BF16)
        xkT = np.ascontiguousarray(key[b].T).astype(NP_BF16)
        xvT = np.ascontiguousarray(value[b].T).astype(NP_BF16)
        keepT = np.ascontiguousarray((~mask[b, 0]).T).astype(NP_BF16)
        per_b.append((xqT, xkT, xvT, keepT))
    per_g = []
    for g in range(4):
        sl = slice(g * DQ, (g + 1) * DQ)
        per_g.append((
            np.ascontiguousarray(Wq[sl].T).astype(NP_BF16),
            np.ascontiguousarray(Wk[sl].T).astype(NP_BF16),
            np.ascontiguousarray(Wv[sl].T).astype(NP_BF16),
            np.ascontiguousarray(Wo[:, sl].T).astype(NP_BF16),
            np.ascontiguousarray(bq[sl].reshape(DQ // P, P).T).astype(np.float32),
            np.ascontiguousarray(bk[sl].reshape(DQ // P, P).T).astype(np.float32),
            np.ascontiguousarray(np.broadcast_to(bv[sl], (P, DQ))).astype(np.float32),
        ))
    in_maps = []
    for c in range(NCORES):
        b, g = c // 4, c % 4
        xqT, xkT, xvT, keepT = per_b[b % nb]
        wqT, wkT, wvT, woT, bq2, bk2, bvr = per_g[g]
        in_maps.append({
            "xq": xqT, "xk": xkT, "xv": xvT,
            "wq": wqT, "wk": wkT, "wv": wvT, "wo": woT,
            "bq": bq2, "bk": bk2, "bv": bvr,
            "keep": keepT,
        })
    return in_maps


def gather_output(results, bo, nb=B, s=S, d=D):
    out = np.empty((nb, s, d), np.float32)
    for b in range(nb):
        acc = results[4 * b]["out"].astype(np.float32)
        for g in range(1, 4):
            acc += results[4 * b + g]["out"].astype(np.float32)
        out[b] = acc.T
    out += bo.astype(np.float32)
    return out


def run_on_cores(in_maps, trace=False, **kw):
    nc = _get_nc()
    return run_bass_kernel_spmd(nc, in_maps, list(range(NCORES)), trace=trace, **kw)


def kernel(query, key, value, mask, Wq, bq, Wk, bk, Wv, bv, Wo, bo):
    in_maps = make_in_maps(query, key, value, mask,
                           Wq, bq, Wk, bk, Wv, bv, Wo, bo)
    res = run_on_cores(in_maps, trace=False)
    return gather_output(res.results, bo)


# revision 26
# speedup vs baseline: 1.1432x; 1.0205x over previous
"""Multi-head attention Bass kernel for Trainium2, sharded over 8 NeuronCores.

Sharding: core c handles batch b = c//4 and head-group g = c%4 (4 of 16 heads,
i.e. a 256-wide slice of the QKV projection output).  Each core computes its
heads' attention and a partial output projection (contribution of its 256
ctx columns to the full [S, D] output).  The host sums the 4 partials per
batch and adds the output bias.

Device-side layout choices:
  - activations shipped pre-transposed: xT = x.T  [D, S] so the contraction
    dim (D) lands on SBUF partitions without any on-device transpose.
  - scores are computed transposed (scoresT[sk, sq]) so the attention weights
    leave softmax with sk on partitions — the contraction layout attn@V needs.
  - softmax denominator comes free from a ones-column appended to V
    (ctx psum row 64 = sum_sk attn);  no max-subtraction (scores bounded).
  - masking is a multiply by a 0/1 bf16 keep-mask after exp.

v2 pipeline (vs the 368us baseline whose PE ran cold and stalled on psum):
  - stage B streams one head x one 1024-wide sq block at a time: score psum
    tiles double-buffered (2x2 banks) so the PE runs ahead of the exp;
    ctx accumulates in a 3rd/4th bank pair.  Score matmuls for one (h, sk)
    share one LDWEIGHTS (j-inner ordering), same for attn@V.
  - ctx psum is released immediately after a raw bf16 copy to SBUF; the
    softmax division (reciprocal+broadcast+multiply) happens out of the
    critical loop.
  - output projection is emitted interleaved with the following sq-block's
    attention so its matmuls fill PE slack; results staged bf16 and the
    host sums partials in fp32.
  - out/ctx copies and V-wave DMAs sliced so xq/xk/xv are each read once.
"""

import numpy as np
import ml_dtypes

import concourse.bass as bass
import concourse.mybir as mybir
import concourse.tile as tile
from concourse import bacc, library_config
from concourse.bass_utils import run_bass_kernel_spmd

# Problem shapes (hardcoded per contest rules).
B, S, D, H, DH = 2, 2048, 1024, 16, 64
NCORES = 8
NH = 4            # heads per core
DQ = NH * DH      # 256: per-core q/k/v width
P = 128

F32 = mybir.dt.float32
BF16 = mybir.dt.bfloat16
NP_BF16 = ml_dtypes.bfloat16

SQC = 1024        # sq block width (exp grain; 2 psum banks)
FDP = 512         # matmul moving free-dim (one fp32 psum bank)


def build_nc(s=S, d=D):
    """Build the per-core Bass program (same NEFF on all 8 cores)."""
    ko = d // P           # 8 contraction chunks for projections
    mq = DQ // P          # 2 q/k partition chunks
    skn = s // P          # 16 sk chunks
    nsq = s // SQC        # 2 sq blocks
    nsf = s // FDP        # 4 projection n-chunks
    nj = SQC // FDP       # 2 matmuls per score tile

    nc = bacc.Bacc("TRN2", debug=False)

    xq_t = nc.declare_dram_parameter("xq", [d, s], BF16, isOutput=False)
    xk_t = nc.declare_dram_parameter("xk", [d, s], BF16, isOutput=False)
    xv_t = nc.declare_dram_parameter("xv", [d, s], BF16, isOutput=False)
    wq_t = nc.declare_dram_parameter("wq", [d, DQ], BF16, isOutput=False)
    wk_t = nc.declare_dram_parameter("wk", [d, DQ], BF16, isOutput=False)
    wv_t = nc.declare_dram_parameter("wv", [d, DQ], BF16, isOutput=False)
    wo_t = nc.declare_dram_parameter("wo", [DQ, d], BF16, isOutput=False)
    bq_t = nc.declare_dram_parameter("bq", [P, mq], F32, isOutput=False)
    bk_t = nc.declare_dram_parameter("bk", [P, mq], F32, isOutput=False)
    bv_t = nc.declare_dram_parameter("bv", [P, DQ], F32, isOutput=False)
    keep_t = nc.declare_dram_parameter("keep", [s, s], BF16, isOutput=False)
    out_t = nc.declare_dram_parameter("out", [d, s], BF16, isOutput=True)

    AF = mybir.ActivationFunctionType
    OP = mybir.AluOpType

    with tile.TileContext(nc) as tc:
        nc.gpsimd.load_library(library_config.attn)
        with (
            tc.tile_pool(name="const", bufs=1) as const,
            tc.tile_pool(name="xs", bufs=8) as xs,
            tc.tile_pool(name="xvp", bufs=3) as xvp,
            tc.tile_pool(name="attn", bufs=4) as attnp,
            tc.tile_pool(name="sc", bufs=2) as scp,
            tc.tile_pool(name="outp", bufs=4) as outp,
            tc.tile_pool(name="pss", bufs=2, space="PSUM") as pss,
            tc.tile_pool(name="psc", bufs=1, space="PSUM") as psc,
            tc.tile_pool(name="pso", bufs=2, space="PSUM") as pso,
        ):
            # ---- persistent SBUF tensors ----
            wq_sb = const.tile([P, ko, DQ], BF16, tag="wq")
            wk_sb = const.tile([P, ko, DQ], BF16, tag="wk")
            wv_sb = const.tile([P, ko, DQ], BF16, tag="wv")
            wo_sb = const.tile([P, mq, d], BF16, tag="wo")
            bq_sb = const.tile([P, mq], F32, tag="bq")
            bk_sb = const.tile([P, mq], F32, tag="bk")
            bv_sb = const.tile([P, DQ], F32, tag="bv")
            qT_sb = const.tile([P, mq, s], BF16, tag="qT")
            kT_sb = const.tile([P, mq, s], BF16, tag="kT")
            v_sb = const.tile([P, skn, NH * 65], BF16, tag="v")
            keep_sb = const.tile([P, skn, s], BF16, tag="keep")
            ctxT_sb = const.tile([P, mq, s], BF16, tag="ctxT")

            nc.sync.dma_start(wq_sb, wq_t[:].rearrange("(ko p) m -> p ko m", p=P))
            nc.sync.dma_start(wk_sb, wk_t[:].rearrange("(ko p) m -> p ko m", p=P))
            nc.sync.dma_start(wv_sb, wv_t[:].rearrange("(ko p) m -> p ko m", p=P))
            nc.sync.dma_start(wo_sb, wo_t[:].rearrange("(mq p) n -> p mq n", p=P))
            nc.sync.dma_start(bq_sb, bq_t[:])
            nc.sync.dma_start(bk_sb, bk_t[:])
            nc.sync.dma_start(bv_sb, bv_t[:])

            # ones column per head in the V tile (softmax denominator trick)
            nc.vector.memset(
                v_sb[:].rearrange("p s (h c) -> p s h c", h=NH)[:, :, :, 64:65], 1.0
            )

            # ---- stage A: projections, m-outer so attention starts after
            # kproj + qproj(m=0); qproj(m=1) and vproj interleave into the
            # first two attention blocks.
            xk_tiles = []
            xq_tiles = []

            def proj_m_pss(x_tiles, x_t, w_sb, b_sb, dst_sb, m, dma):
                """One m-chunk of a projection, 4 groups in 2 pss tiles."""
                tls = [pss.tile([P, SQC], F32, name="pjA", tag="s"),
                       pss.tile([P, SQC], F32, name="pjB", tag="s")]
                for kk in range(ko):
                    if dma:
                        t = xs.tile([P, s], BF16, tag="xt")
                        nc.sync.dma_start(t, x_t[kk * P:(kk + 1) * P, :])
                        x_tiles.append(t)
                    for n in range(nsf):
                        nc.tensor.matmul(
                            tls[n // nj][:, (n % nj) * FDP:(n % nj + 1) * FDP],
                            w_sb[:, kk, m * P:(m + 1) * P],
                            x_tiles[kk][:, n * FDP:(n + 1) * FDP],
                            start=(kk == 0),
                            stop=(kk == ko - 1),
                        )
                for n in range(nsf):
                    nc.vector.tensor_scalar_add(
                        dst_sb[:, m, n * FDP:(n + 1) * FDP],
                        tls[n // nj][:, (n % nj) * FDP:(n % nj + 1) * FDP],
                        b_sb[:, m:m + 1],
                    )

            def qproj_m1_phase(ph):
                """Half of qproj m=1 (2 n-groups) using pso tiles."""
                tls = [pso.tile([P, FDP], F32, name="qpA", tag="o"),
                       pso.tile([P, FDP], F32, name="qpB", tag="o")]
                for kk in range(ko):
                    for i in range(2):
                        n = 2 * ph + i
                        nc.tensor.matmul(
                            tls[i],
                            wq_sb[:, kk, P:2 * P],
                            xq_tiles[kk][:, n * FDP:(n + 1) * FDP],
                            start=(kk == 0),
                            stop=(kk == ko - 1),
                        )
                for i in range(2):
                    n = 2 * ph + i
                    nc.vector.tensor_scalar_add(
                        qT_sb[:, 1, n * FDP:(n + 1) * FDP],
                        tls[i],
                        bq_sb[:, 1:2],
                    )

            # v projection wave: 2 sv chunks, one batched xv DMA per wave
            v_strided = v_sb[:].rearrange("p s (h c) -> p s h c", h=NH)
            xv_re = xv_t[:].rearrange("(ko p) s -> p ko s", p=P)

            def vproj_wave(w):
                vts = [pso.tile([P, FDP], F32, name="vtA", tag="o"),
                       pso.tile([P, FDP], F32, name="vtB", tag="o")]
                t = xvp.tile([P, ko, 2 * P], BF16, tag="xv")
                nc.sync.dma_start(t, xv_re[:, :, w * 2 * P:(w + 1) * 2 * P])
                for kk in range(ko):
                    for i in range(2):
                        nc.tensor.matmul(
                            vts[i][:, :DQ],
                            t[:, kk, i * P:(i + 1) * P],
                            wv_sb[:, kk, :],
                            start=(kk == 0),
                            stop=(kk == ko - 1),
                        )
                for i in range(2):
                    sv = w * 2 + i
                    nc.vector.tensor_tensor(
                        v_strided[:, sv, :, 0:64],
                        vts[i][:, :DQ].rearrange("p (h c) -> p h c", h=NH),
                        bv_sb[:].rearrange("p (h c) -> p h c", h=NH),
                        OP.add,
                    )

            with nc.named_scope("kproj"):
                proj_m_pss(xk_tiles, xk_t, wk_sb, bk_sb, kT_sb, 0, dma=True)
            # keep-mask streams on the scalar HWDGE queue, in parallel with
            # the x streams on sync; block 0 consumes chunks in sk order.
            for c in range(skn):
                nc.scalar.dma_start(keep_sb[:, c, :], keep_t[c * P:(c + 1) * P, :])
            with nc.named_scope("kproj1"):
                proj_m_pss(xk_tiles, xk_t, wk_sb, bk_sb, kT_sb, 1, dma=False)
            with nc.named_scope("qproj"):
                proj_m_pss(xq_tiles, xq_t, wq_sb, bq_sb, qT_sb, 0, dma=True)

            # ---- out-projection emitter (one do-chunk = one psum tile) ----
            def emit_outproj(sqh, do):
                sq0 = sqh * SQC
                pts = [pso.tile([P, FDP], F32, name="op0", tag="o"),
                       pso.tile([P, FDP], F32, name="op1", tag="o")]
                for kk in range(mq):
                    for n in range(nj):
                        nc.tensor.matmul(
                            pts[n],
                            wo_sb[:, kk, do * P:(do + 1) * P],
                            ctxT_sb[:, kk, sq0 + n * FDP:sq0 + (n + 1) * FDP],
                            start=(kk == 0),
                            stop=(kk == mq - 1),
                        )
                for n in range(nj):
                    ot = outp.tile([P, FDP], BF16, tag="ot")
                    nc.vector.tensor_copy(ot, pts[n])
                    nc.sync.dma_start(
                        out_t[do * P:(do + 1) * P,
                              sq0 + n * FDP:sq0 + (n + 1) * FDP],
                        ot,
                    )

            # ---- stage B: attention, one (sq block, head) per ctx psum ----
            # interleave hooks: block 0 emits a vproj wave before every even
            # sk (v[sk] ready just in time for attn@V); block 1 emits the two
            # qproj m=1 phases (needed by block 2).
            def b0_hook(sk):
                if sk % 2 == 0:
                    vproj_wave(sk // 2)

            def b1_hook(sk):
                if sk == 0:
                    qproj_m1_phase(0)
                elif sk == 4:
                    qproj_m1_phase(1)

            for blk in range(nsq * NH):
                sqh, h = divmod(blk, NH)
                sq0 = sqh * SQC
                hb, hm = (h % 2) * 64, h // 2
                hook = b0_hook if blk == 0 else (b1_hook if blk == 1 else None)
                with nc.named_scope(f"attn{blk}"):
                    cps = psc.tile([P, SQC], F32, name="cps", tag="c")[:65, :]
                    for sk in range(skn):
                        if hook is not None:
                            hook(sk)
                        sps = pss.tile([P, SQC], F32, tag="s")
                        for j in range(nj):
                            nc.tensor.matmul(
                                sps[:, j * FDP:(j + 1) * FDP],
                                kT_sb[hb:hb + 64, hm, sk * P:(sk + 1) * P],
                                qT_sb[hb:hb + 64, hm,
                                      sq0 + j * FDP:sq0 + (j + 1) * FDP],
                                start=True,
                                stop=True,
                            )
                        at = attnp.tile([P, SQC], BF16, tag="at")
                        nc.scalar.activation(at, sps, AF.Exp, scale=0.125)
                        nc.vector.tensor_tensor(
                            at, at, keep_sb[:, sk, sq0:sq0 + SQC], OP.mult)
                        for j in range(nj):
                            nc.tensor.matmul(
                                cps[:, j * FDP:(j + 1) * FDP],
                                v_sb[:, sk, h * 65:(h + 1) * 65],
                                at[:, j * FDP:(j + 1) * FDP],
                                start=(sk == 0),
                                stop=(sk == skn - 1),
                            )
                    # raw ctx + den out of psum fast; normalize off-critical
                    aug = scp.tile([64, SQC], BF16, tag="aug")
                    nc.vector.tensor_copy(aug, cps[0:64, :])
                    dsb = scp.tile([65, SQC], F32, tag="dsb")
                    nc.vector.tensor_copy(dsb[64:65, :], cps[64:65, :])
                    den0 = scp.tile([1, SQC], F32, tag="den0")
                    nc.sync.dma_start(den0, dsb[64:65, :])
                    nc.vector.reciprocal_approx_fast(out=den0, in_=den0)
                    scl = scp.tile([64, SQC], F32, tag="scl")
                    nc.gpsimd.partition_broadcast(scl, den0[0:1, :])
                    cn = scp.tile([64, SQC], BF16, tag="cn")
                    nc.vector.tensor_tensor(cn, aug, scl, OP.mult)
                    nc.sync.dma_start(
                        ctxT_sb[hb:hb + 64, hm, sq0:sq0 + SQC], cn)
                # interleave previous sq block's output projection
                if sqh == nsq - 1:
                    with nc.named_scope("oproj0"):
                        for do in range(h * ko // NH, (h + 1) * ko // NH):
                            emit_outproj(0, do)
            with nc.named_scope("oproj1"):
                for do in range(ko):
                    emit_outproj(1, do)
    nc.compile()
    return nc


_NC_CACHE = {}


def _get_nc(s=S, d=D):
    key = (s, d, SQC)
    if key not in _NC_CACHE:
        _NC_CACHE[key] = build_nc(s, d)
    return _NC_CACHE[key]


def make_in_maps(query, key, value, mask, Wq, bq, Wk, bk, Wv, bv, Wo, bo,
                 s=S, d=D):
    """Build the 8 per-core input maps (host-side shard + layout prep)."""
    nb = query.shape[0]
    per_b = []
    for b in range(nb):
        xqT = np.ascontiguousarray(query[b].T).astype(NP_BF16)
        xkT = np.ascontiguousarray(key[b].T).astype(NP_BF16)
        xvT = np.ascontiguousarray(value[b].T).astype(NP_BF16)
        keepT = np.ascontiguousarray((~mask[b, 0]).T).astype(NP_BF16)
        per_b.append((xqT, xkT, xvT, keepT))
    per_g = []
    for g in range(4):
        sl = slice(g * DQ, (g + 1) * DQ)
        per_g.append((
            np.ascontiguousarray(Wq[sl].T).astype(NP_BF16),
            np.ascontiguousarray(Wk[sl].T).astype(NP_BF16),
            np.ascontiguousarray(Wv[sl].T).astype(NP_BF16),
            np.ascontiguousarray(Wo[:, sl].T).astype(NP_BF16),
            np.ascontiguousarray(bq[sl].reshape(DQ // P, P).T).astype(np.float32),
            np.ascontiguousarray(bk[sl].reshape(DQ // P, P).T).astype(np.float32),
            np.ascontiguousarray(np.broadcast_to(bv[sl], (P, DQ))).astype(np.float32),
        ))
    in_maps = []
    for c in range(NCORES):
        b, g = c // 4, c % 4
        xqT, xkT, xvT, keepT = per_b[b % nb]
        wqT, wkT, wvT, woT, bq2, bk2, bvr = per_g[g]
        in_maps.append({
            "xq": xqT, "xk": xkT, "xv": xvT,
            "wq": wqT, "wk": wkT, "wv": wvT, "wo": woT,
            "bq": bq2, "bk": bk2, "bv": bvr,
            "keep": keepT,
        })
    return in_maps


def gather_output(results, bo, nb=B, s=S, d=D):
    out = np.empty((nb, s, d), np.float32)
    for b in range(nb):
        acc = results[4 * b]["out"].astype(np.float32)
        for g in range(1, 4):
            acc += results[4 * b + g]["out"].astype(np.float32)
        out[b] = acc.T
    out += bo.astype(np.float32)
    return out


def run_on_cores(in_maps, trace=False, **kw):
    nc = _get_nc()
    return run_bass_kernel_spmd(nc, in_maps, list(range(NCORES)), trace=trace, **kw)


def kernel(query, key, value, mask, Wq, bq, Wk, bk, Wv, bv, Wo, bo):
    in_maps = make_in_maps(query, key, value, mask,
                           Wq, bq, Wk, bk, Wv, bv, Wo, bo)
    res = run_on_cores(in_maps, trace=False)
    return gather_output(res.results, bo)


# revision 31
# speedup vs baseline: 1.2343x; 1.0797x over previous
"""Multi-head attention Bass kernel for Trainium2, sharded over 8 NeuronCores.

Sharding: core c handles batch b = c//4 and head-group g = c%4 (4 of 16 heads,
i.e. a 256-wide slice of the QKV projection output).  Each core computes its
heads' attention and a partial output projection (contribution of its 256
ctx columns to the full [S, D] output).  The host sums the 4 partials per
batch and adds the output bias.

Device-side layout choices:
  - activations shipped pre-transposed: xT = x.T  [D, S] so the contraction
    dim (D) lands on SBUF partitions without any on-device transpose.
  - scores are computed transposed (scoresT[sk, sq]) so the attention weights
    leave softmax with sk on partitions — the contraction layout attn@V needs.
  - softmax denominator comes free from a ones-column appended to V
    (ctx psum row 64 = sum_sk attn);  no max-subtraction (scores bounded).
  - masking is a multiply by a 0/1 bf16 keep-mask after exp.

v2 pipeline (vs the 368us baseline whose PE ran cold and stalled on psum):
  - stage B streams one head x one 1024-wide sq block at a time: score psum
    tiles double-buffered (2x2 banks) so the PE runs ahead of the exp;
    ctx accumulates in a 3rd/4th bank pair.  Score matmuls for one (h, sk)
    share one LDWEIGHTS (j-inner ordering), same for attn@V.
  - ctx psum is released immediately after a raw bf16 copy to SBUF; the
    softmax division (reciprocal+broadcast+multiply) happens out of the
    critical loop.
  - output projection is emitted interleaved with the following sq-block's
    attention so its matmuls fill PE slack; results staged bf16 and the
    host sums partials in fp32.
  - out/ctx copies and V-wave DMAs sliced so xq/xk/xv are each read once.
"""

import numpy as np
import ml_dtypes

import concourse.bass as bass
import concourse.mybir as mybir
import concourse.tile as tile
from concourse import bacc, library_config
from concourse.bass_utils import run_bass_kernel_spmd

# Problem shapes (hardcoded per contest rules).
B, S, D, H, DH = 2, 2048, 1024, 16, 64
NCORES = 8
NH = 4            # heads per core
DQ = NH * DH      # 256: per-core q/k/v width
P = 128

F32 = mybir.dt.float32
BF16 = mybir.dt.bfloat16
NP_BF16 = ml_dtypes.bfloat16

SQC = 1024        # sq block width (exp grain; 2 psum banks)
FDP = 512         # matmul moving free-dim (one fp32 psum bank)


def build_nc(s=S, d=D):
    """Build the per-core Bass program (same NEFF on all 8 cores)."""
    ko = d // P           # 8 contraction chunks for projections
    mq = DQ // P          # 2 q/k partition chunks
    skn = s // P          # 16 sk chunks
    nsq = s // SQC        # 2 sq blocks
    nsf = s // FDP        # 4 projection n-chunks
    nj = SQC // FDP       # 2 matmuls per score tile

    nc = bacc.Bacc("TRN2", debug=False)

    xq_t = nc.declare_dram_parameter("xq", [d, s], BF16, isOutput=False)
    xk_t = nc.declare_dram_parameter("xk", [d, s], BF16, isOutput=False)
    xv_t = nc.declare_dram_parameter("xv", [d, s], BF16, isOutput=False)
    wq_t = nc.declare_dram_parameter("wq", [d, DQ], BF16, isOutput=False)
    wk_t = nc.declare_dram_parameter("wk", [d, DQ], BF16, isOutput=False)
    wv_t = nc.declare_dram_parameter("wv", [d, DQ], BF16, isOutput=False)
    wo_t = nc.declare_dram_parameter("wo", [DQ, d], BF16, isOutput=False)
    bq_t = nc.declare_dram_parameter("bq", [P, mq], F32, isOutput=False)
    bk_t = nc.declare_dram_parameter("bk", [P, mq], F32, isOutput=False)
    bv_t = nc.declare_dram_parameter("bv", [P, DQ], F32, isOutput=False)
    keep_t = nc.declare_dram_parameter("keep", [s, s], BF16, isOutput=False)
    out_t = nc.declare_dram_parameter("out", [d, s], BF16, isOutput=True)

    AF = mybir.ActivationFunctionType
    OP = mybir.AluOpType

    with tile.TileContext(nc) as tc:
        nc.gpsimd.load_library(library_config.attn)
        with (
            tc.tile_pool(name="const", bufs=1) as const,
            tc.tile_pool(name="xs", bufs=8) as xs,
            tc.tile_pool(name="xvp", bufs=3) as xvp,
            tc.tile_pool(name="attn", bufs=6) as attnp,
            tc.tile_pool(name="sc", bufs=2) as scp,
            tc.tile_pool(name="outp", bufs=4) as outp,
            tc.tile_pool(name="pss", bufs=2, space="PSUM") as pss,
            tc.tile_pool(name="psc", bufs=1, space="PSUM") as psc,
            tc.tile_pool(name="pso", bufs=2, space="PSUM") as pso,
        ):
            # ---- persistent SBUF tensors ----
            wq_sb = const.tile([P, ko, DQ], BF16, tag="wq")
            wk_sb = const.tile([P, ko, DQ], BF16, tag="wk")
            wv_sb = const.tile([P, ko, DQ], BF16, tag="wv")
            wo_sb = const.tile([P, mq, d], BF16, tag="wo")
            bq_sb = const.tile([P, mq], F32, tag="bq")
            bk_sb = const.tile([P, mq], F32, tag="bk")
            bv_sb = const.tile([P, DQ], F32, tag="bv")
            qT_sb = const.tile([P, mq, s], BF16, tag="qT")
            kT_sb = const.tile([P, mq, s], BF16, tag="kT")
            v_sb = const.tile([P, skn, NH * 65], BF16, tag="v")
            keep_sb = const.tile([P, skn, s], BF16, tag="keep")
            ctxT_sb = const.tile([P, mq, s], BF16, tag="ctxT")

            # only wk gates the first matmul; heavier weight DMAs ride later
            nc.sync.dma_start(wk_sb, wk_t[:].rearrange("(ko p) m -> p ko m", p=P))
            nc.scalar.dma_start(bk_sb, bk_t[:])
            nc.scalar.dma_start(bq_sb, bq_t[:])
            nc.scalar.dma_start(bv_sb, bv_t[:])

            # ones column per head in the V tile (softmax denominator trick)
            nc.vector.memset(
                v_sb[:].rearrange("p s (h c) -> p s h c", h=NH)[:, :, :, 64:65], 1.0
            )

            # ---- stage A: projections, m-outer so attention starts after
            # kproj + qproj(m=0); qproj(m=1) and vproj interleave into the
            # first two attention blocks.
            xk_tiles = []
            xq_tiles = []

            def proj_m_pss(x_tiles, x_t, w_sb, b_sb, dst_sb, m, dma):
                """One m-chunk of a projection, 4 groups in 2 pss tiles."""
                tls = [pss.tile([P, SQC], F32, name="pjA", tag="s"),
                       pss.tile([P, SQC], F32, name="pjB", tag="s")]
                for kk in range(ko):
                    if dma:
                        t = xs.tile([P, s], BF16, tag="xt")
                        nc.sync.dma_start(t, x_t[kk * P:(kk + 1) * P, :])
                        x_tiles.append(t)
                    for n in range(nsf):
                        nc.tensor.matmul(
                            tls[n // nj][:, (n % nj) * FDP:(n % nj + 1) * FDP],
                            w_sb[:, kk, m * P:(m + 1) * P],
                            x_tiles[kk][:, n * FDP:(n + 1) * FDP],
                            start=(kk == 0),
                            stop=(kk == ko - 1),
                        )
                for n in range(nsf):
                    nc.vector.tensor_scalar_add(
                        dst_sb[:, m, n * FDP:(n + 1) * FDP],
                        tls[n // nj][:, (n % nj) * FDP:(n % nj + 1) * FDP],
                        b_sb[:, m:m + 1],
                    )

            def qproj_m1_phase(ph):
                """Half of qproj m=1 (2 n-groups) using pso tiles."""
                tls = [pso.tile([P, FDP], F32, name="qpA", tag="o"),
                       pso.tile([P, FDP], F32, name="qpB", tag="o")]
                for kk in range(ko):
                    for i in range(2):
                        n = 2 * ph + i
                        nc.tensor.matmul(
                            tls[i],
                            wq_sb[:, kk, P:2 * P],
                            xq_tiles[kk][:, n * FDP:(n + 1) * FDP],
                            start=(kk == 0),
                            stop=(kk == ko - 1),
                        )
                for i in range(2):
                    n = 2 * ph + i
                    nc.vector.tensor_scalar_add(
                        qT_sb[:, 1, n * FDP:(n + 1) * FDP],
                        tls[i],
                        bq_sb[:, 1:2],
                    )

            # v projection wave: 2 sv chunks, one batched xv DMA per wave
            v_strided = v_sb[:].rearrange("p s (h c) -> p s h c", h=NH)
            xv_re = xv_t[:].rearrange("(ko p) s -> p ko s", p=P)

            def vproj_wave(w):
                vts = [pso.tile([P, FDP], F32, name="vtA", tag="o"),
                       pso.tile([P, FDP], F32, name="vtB", tag="o")]
                t = xvp.tile([P, ko, 2 * P], BF16, tag="xv")
                nc.sync.dma_start(t, xv_re[:, :, w * 2 * P:(w + 1) * 2 * P])
                for kk in range(ko):
                    for i in range(2):
                        nc.tensor.matmul(
                            vts[i][:, :DQ],
                            t[:, kk, i * P:(i + 1) * P],
                            wv_sb[:, kk, :],
                            start=(kk == 0),
                            stop=(kk == ko - 1),
                        )
                for i in range(2):
                    sv = w * 2 + i
                    nc.vector.tensor_tensor(
                        v_strided[:, sv, :, 0:64],
                        vts[i][:, :DQ].rearrange("p (h c) -> p h c", h=NH),
                        bv_sb[:].rearrange("p (h c) -> p h c", h=NH),
                        OP.add,
                    )

            with nc.named_scope("kproj"):
                proj_m_pss(xk_tiles, xk_t, wk_sb, bk_sb, kT_sb, 0, dma=True)
            # keep-mask streams on the scalar HWDGE queue, in parallel with
            # the x streams on sync; block 0 consumes chunks in sk order.
            for c in range(skn):
                nc.scalar.dma_start(keep_sb[:, c, :], keep_t[c * P:(c + 1) * P, :])
            nc.sync.dma_start(wq_sb, wq_t[:].rearrange("(ko p) m -> p ko m", p=P))
            with nc.named_scope("kproj1"):
                proj_m_pss(xk_tiles, xk_t, wk_sb, bk_sb, kT_sb, 1, dma=False)
            with nc.named_scope("qproj"):
                proj_m_pss(xq_tiles, xq_t, wq_sb, bq_sb, qT_sb, 0, dma=True)
            nc.sync.dma_start(wv_sb, wv_t[:].rearrange("(ko p) m -> p ko m", p=P))
            nc.sync.dma_start(wo_sb, wo_t[:].rearrange("(mq p) n -> p mq n", p=P))

            # ---- out-projection emitter (one do-chunk = one psum tile) ----
            def emit_outproj(sqh, do):
                sq0 = sqh * SQC
                pts = [pso.tile([P, FDP], F32, name="op0", tag="o"),
                       pso.tile([P, FDP], F32, name="op1", tag="o")]
                for kk in range(mq):
                    for n in range(nj):
                        nc.tensor.matmul(
                            pts[n],
                            wo_sb[:, kk, do * P:(do + 1) * P],
                            ctxT_sb[:, kk, sq0 + n * FDP:sq0 + (n + 1) * FDP],
                            start=(kk == 0),
                            stop=(kk == mq - 1),
                        )
                for n in range(nj):
                    ot = outp.tile([P, FDP], BF16, tag="ot")
                    nc.vector.tensor_copy(ot, pts[n])
                    nc.sync.dma_start(
                        out_t[do * P:(do + 1) * P,
                              sq0 + n * FDP:sq0 + (n + 1) * FDP],
                        ot,
                    )

            # ---- stage B: attention, one (sq block, head) per ctx psum ----
            # interleave hooks: block 0 emits a vproj wave before every even
            # sk (v[sk] ready just in time for attn@V); block 1 emits the two
            # qproj m=1 phases (needed by block 2).
            def b0_hook(sk):
                if sk % 2 == 0:
                    vproj_wave(sk // 2)

            def b1_hook(sk):
                if sk == 0:
                    qproj_m1_phase(0)

            def b2_hook(sk):
                if sk == 0:
                    qproj_m1_phase(1)

            for blk in range(nsq * NH):
                sqh, h = divmod(blk, NH)
                sq0 = sqh * SQC
                hb, hm = (h % 2) * 64, h // 2
                hook = {0: b0_hook, 1: b1_hook, 2: b2_hook}.get(blk)
                with nc.named_scope(f"attn{blk}"):
                    cps = psc.tile([P, SQC], F32, name="cps", tag="c")[:65, :]
                    for sk in range(skn):
                        if hook is not None:
                            hook(sk)
                        sps = pss.tile([P, SQC], F32, tag="s")
                        for j in range(nj):
                            nc.tensor.matmul(
                                sps[:, j * FDP:(j + 1) * FDP],
                                kT_sb[hb:hb + 64, hm, sk * P:(sk + 1) * P],
                                qT_sb[hb:hb + 64, hm,
                                      sq0 + j * FDP:sq0 + (j + 1) * FDP],
                                start=True,
                                stop=True,
                            )
                        at = attnp.tile([P, SQC], BF16, tag="at")
                        nc.scalar.activation(at, sps, AF.Exp, scale=0.125)
                        nc.vector.tensor_tensor(
                            at, at, keep_sb[:, sk, sq0:sq0 + SQC], OP.mult)
                        for j in range(nj):
                            nc.tensor.matmul(
                                cps[:, j * FDP:(j + 1) * FDP],
                                v_sb[:, sk, h * 65:(h + 1) * 65],
                                at[:, j * FDP:(j + 1) * FDP],
                                start=(sk == 0),
                                stop=(sk == skn - 1),
                            )
                    # raw ctx + den out of psum fast; normalize off-critical
                    aug = scp.tile([64, SQC], BF16, tag="aug")
                    nc.vector.tensor_copy(aug, cps[0:64, :])
                    dsb = scp.tile([65, SQC], F32, tag="dsb")
                    nc.vector.tensor_copy(dsb[64:65, :], cps[64:65, :])
                    den0 = scp.tile([1, SQC], F32, tag="den0")
                    nc.sync.dma_start(den0, dsb[64:65, :])
                    nc.vector.reciprocal_approx_fast(out=den0, in_=den0)
                    scl = scp.tile([64, SQC], F32, tag="scl")
                    nc.gpsimd.partition_broadcast(scl, den0[0:1, :])
                    cn = scp.tile([64, SQC], BF16, tag="cn")
                    nc.vector.tensor_tensor(cn, aug, scl, OP.mult)
                    nc.sync.dma_start(
                        ctxT_sb[hb:hb + 64, hm, sq0:sq0 + SQC], cn)
                # interleave previous sq block's output projection
                if sqh == nsq - 1:
                    with nc.named_scope("oproj0"):
                        for do in range(h * ko // NH, (h + 1) * ko // NH):
                            emit_outproj(0, do)
            with nc.named_scope("oproj1"):
                for do in range(ko):
                    emit_outproj(1, do)
    nc.compile()
    return nc


_NC_CACHE = {}


def _get_nc(s=S, d=D):
    key = (s, d, SQC)
    if key not in _NC_CACHE:
        _NC_CACHE[key] = build_nc(s, d)
    return _NC_CACHE[key]


def make_in_maps(query, key, value, mask, Wq, bq, Wk, bk, Wv, bv, Wo, bo,
                 s=S, d=D):
    """Build the 8 per-core input maps (host-side shard + layout prep)."""
    nb = query.shape[0]
    per_b = []
    for b in range(nb):
        xqT = np.ascontiguousarray(query[b].T).astype(NP_BF16)
        xkT = np.ascontiguousarray(key[b].T).astype(NP_BF16)
        xvT = np.ascontiguousarray(value[b].T).astype(NP_BF16)
        keepT = np.ascontiguousarray((~mask[b, 0]).T).astype(NP_BF16)
        per_b.append((xqT, xkT, xvT, keepT))
    per_g = []
    for g in range(4):
        sl = slice(g * DQ, (g + 1) * DQ)
        per_g.append((
            np.ascontiguousarray(Wq[sl].T).astype(NP_BF16),
            np.ascontiguousarray(Wk[sl].T).astype(NP_BF16),
            np.ascontiguousarray(Wv[sl].T).astype(NP_BF16),
            np.ascontiguousarray(Wo[:, sl].T).astype(NP_BF16),
            np.ascontiguousarray(bq[sl].reshape(DQ // P, P).T).astype(np.float32),
            np.ascontiguousarray(bk[sl].reshape(DQ // P, P).T).astype(np.float32),
            np.ascontiguousarray(np.broadcast_to(bv[sl], (P, DQ))).astype(np.float32),
        ))
    in_maps = []
    for c in range(NCORES):
        b, g = c // 4, c % 4
        xqT, xkT, xvT, keepT = per_b[b % nb]
        wqT, wkT, wvT, woT, bq2, bk2, bvr = per_g[g]
        in_maps.append({
            "xq": xqT, "xk": xkT, "xv": xvT,
            "wq": wqT, "wk": wkT, "wv": wvT, "wo": woT,
            "bq": bq2, "bk": bk2, "bv": bvr,
            "keep": keepT,
        })
    return in_maps


def gather_output(results, bo, nb=B, s=S, d=D):
    out = np.empty((nb, s, d), np.float32)
    for b in range(nb):
        acc = results[4 * b]["out"].astype(np.float32)
        for g in range(1, 4):
            acc += results[4 * b + g]["out"].astype(np.float32)
        out[b] = acc.T
    out += bo.astype(np.float32)
    return out


def run_on_cores(in_maps, trace=False, **kw):
    nc = _get_nc()
    return run_bass_kernel_spmd(nc, in_maps, list(range(NCORES)), trace=trace, **kw)


def kernel(query, key, value, mask, Wq, bq, Wk, bk, Wv, bv, Wo, bo):
    in_maps = make_in_maps(query, key, value, mask,
                           Wq, bq, Wk, bk, Wv, bv, Wo, bo)
    res = run_on_cores(in_maps, trace=False)
    return gather_output(res.results, bo)
